# revision 9
# baseline (speedup 1.0000x reference)
import os
import sys

for _p in ("/opt/trn_rl_repo", "/root/.axon_site/_ro/trn_rl_repo"):
    if os.path.isdir(_p) and _p not in sys.path:
        sys.path.insert(0, _p)

import numpy as np


def _ensure_ntff_hook():
    """bass_utils hard-imports antenv.axon_hooks when BASS_TRACE is set.
    The agent image ships an antenv stub without it; register the same
    ctypes-based hook trn_boot would have installed."""
    try:
        import antenv.axon_hooks  # noqa: F401
        return
    except ImportError:
        pass
    import types
    import ctypes
    import contextlib

    mod = types.ModuleType("antenv.axon_hooks")
    state = {"hook": None}

    def set_axon_ntff_profile_hook(h):
        state["hook"] = h

    def get_axon_ntff_profile_hook():
        return state["hook"]

    mod.set_axon_ntff_profile_hook = set_axon_ntff_profile_hook
    mod.get_axon_ntff_profile_hook = get_axon_ntff_profile_hook

    so_path = "/opt/axon/libaxon_pjrt.so"
    try:
        lib = ctypes.CDLL(so_path)
        if hasattr(lib, "axon_start_nrt_profile"):
            lib.axon_start_nrt_profile.argtypes = [
                ctypes.POINTER(ctypes.c_int64),
                ctypes.c_size_t,
            ]
            lib.axon_start_nrt_profile.restype = ctypes.c_int64
            lib.axon_stop_nrt_profile.argtypes = [ctypes.c_char_p]
            lib.axon_stop_nrt_profile.restype = ctypes.c_int64

            @contextlib.contextmanager
            def _hook(output_dir, device_ids):
                import jax

                jax.devices()
                if device_ids:
                    ids = (ctypes.c_int64 * len(device_ids))(*device_ids)
                    rc = lib.axon_start_nrt_profile(ids, len(device_ids))
                else:
                    rc = lib.axon_start_nrt_profile(None, 0)
                if rc != 0:
                    raise RuntimeError(f"axon_start_nrt_profile rc={rc}")
                try:
                    yield
                finally:
                    n = lib.axon_stop_nrt_profile(str(output_dir).encode())
                    if n < 0:
                        raise RuntimeError(f"axon_stop_nrt_profile rc={n}")

            state["hook"] = _hook
    except OSError:
        pass

    import antenv

    antenv.axon_hooks = mod
    sys.modules["antenv.axon_hooks"] = mod


_ensure_ntff_hook()

N_CORES = 8
T_FULL = 16384
T = T_FULL // N_CORES      # 2048 tokens per core
D = 7168
E = 256
KT = D // 128              # 56 contraction tiles
NT = T // 128              # 16 token tiles per core
NST = 8                    # super-tiles of 256 tokens
KC = 14                    # k-tiles per DMA chunk
NCHUNK = KT // KC          # 4 chunks per super-tile

N_GROUPS = 8
GROUP_SIZE = E // N_GROUPS  # 32
TOPK = 8
ROUTE_SCALE = 2.5
SHIFT = 4.0                # group-select shift: separates selected groups
EPS = 2.0 ** -11           # bias-encoding scale for the second top-8 pass

_NC = None


def _build_nc():
    import concourse.bass as bass
    import concourse.tile as tile
    from concourse import bacc, mybir

    nc = bacc.Bacc(None, target_bir_lowering=False)
    f32 = mybir.dt.float32
    f16 = mybir.dt.float16
    u16 = mybir.dt.uint16
    u32 = mybir.dt.uint32
    Alu = mybir.AluOpType
    AX = mybir.AxisListType
    Act = mybir.ActivationFunctionType

    # host-packed layouts (see kernel()):
    #   xp[st*128+p, k*256+c] = x[token st*256+c, dim k*128+p]  (fp16)
    #   wp[p, k*256+e]        = weight[e, k*128+p]              (fp16)
    xp = nc.dram_tensor("xp", [NST * 128, KT * 256], f16, kind="ExternalInput")
    wp = nc.dram_tensor("wp", [128, KT * 256], f16, kind="ExternalInput")
    bb = nc.dram_tensor("bb", [128, E], f32, kind="ExternalInput")
    be = nc.dram_tensor("be", [128, E], f32, kind="ExternalInput")  # bias * EPS
    wout = nc.dram_tensor("wout", [128, NT * TOPK], f32, kind="ExternalOutput")
    iout = nc.dram_tensor("iout", [128, NT * TOPK], u32, kind="ExternalOutput")

    with tile.TileContext(nc) as tc:
        with (
            tc.tile_pool(name="const", bufs=1) as cpool,
            tc.tile_pool(name="xc", bufs=3 * NCHUNK) as xpool,
            tc.tile_pool(name="rt", bufs=3) as rpool,
            tc.tile_pool(name="sm", bufs=3) as spool,
            tc.tile_pool(name="ps", bufs=4, space=bass.MemorySpace.PSUM) as pspool,
        ):
            # resident gate weight [128, KT*256] fp16, loaded in 4 chunks
            wsb = cpool.tile([128, KT * 256], f16)
            bbs = cpool.tile([128, E], f32)
            bes = cpool.tile([128, E], f32)

            # persistent accumulators
            sig_top = cpool.tile([128, NT, TOPK], f32)
            iraw = cpool.tile([128, NT, TOPK], u16)

            for st in range(NST):
                chunks = []
                for c in range(NCHUNK):
                    if st == 0:
                        # w chunks on the Sync HWDGE queue, x chunks on the
                        # Scalar queue: descriptor generation (~0.7us each)
                        # runs in parallel and the first w/x pair lands first
                        nc.sync.dma_start(
                            wsb[:, c * KC * 256:(c + 1) * KC * 256],
                            wp[:, c * KC * 256:(c + 1) * KC * 256],
                        )
                    xc = xpool.tile([128, KC * 256], f16, tag="xc")
                    nc.scalar.dma_start(
                        xc[:],
                        xp[st * 128:(st + 1) * 128, c * KC * 256:(c + 1) * KC * 256],
                    )
                    chunks.append(xc)
                if st == 0:
                    nc.sync.dma_start(bbs[:], bb[:, :])
                    nc.sync.dma_start(bes[:], be[:, :])

                for tt in range(2):
                    t = st * 2 + tt
                    ps = pspool.tile([128, E], f32)
                    for k in range(KT):
                        c, kk = divmod(k, KC)
                        off = kk * 256 + tt * 128
                        nc.tensor.matmul(
                            ps[:],
                            chunks[c][:, off:off + 128],
                            wsb[:, k * 256:(k + 1) * 256],
                            start=(k == 0),
                            stop=(k == KT - 1),
                        )

                    # ---- routing for this 128-token tile ----
                    sig = rpool.tile([128, E], f32, tag="sig")
                    nc.scalar.activation(sig[:], ps[:], Act.Sigmoid)
                    s = rpool.tile([128, E], f32, tag="s")
                    nc.gpsimd.tensor_tensor(s[:], sig[:], bbs[:], op=Alu.add)
                    sv = s[:].rearrange("p (g e) -> p g e", g=N_GROUPS)

                    m1 = spool.tile([128, N_GROUPS], f32, tag="m1")
                    nc.vector.reduce_max(m1[:], sv, axis=AX.X)
                    mr = rpool.tile([128, E], f32, tag="mr")
                    nc.vector.match_replace(mr[:], m1[:], s[:], -1e30)
                    m2 = spool.tile([128, N_GROUPS], f32, tag="m2")
                    nc.vector.reduce_max(
                        m2[:], mr[:].rearrange("p (g e) -> p g e", g=N_GROUPS),
                        axis=AX.X,
                    )
                    gs = spool.tile([128, N_GROUPS], f32, tag="gs")
                    nc.vector.tensor_tensor(gs[:], m1[:], m2[:], op=Alu.add)
                    srt = spool.tile([128, N_GROUPS], f32, tag="srt")
                    nc.vector.max(srt[:], gs[:])
                    # keep4 = (gs >= 4th-largest) * SHIFT
                    keep4 = spool.tile([128, N_GROUPS], f32, tag="keep4")
                    nc.vector.tensor_scalar(
                        keep4[:], gs[:], srt[:, 3:4], SHIFT,
                        op0=Alu.is_ge, op1=Alu.mult,
                    )
                    sshift = rpool.tile([128, E], f32, tag="sshift")
                    nc.gpsimd.tensor_tensor(
                        sshift[:].rearrange("p (g e) -> p g e", g=N_GROUPS),
                        keep4[:].broadcast_to([128, N_GROUPS, GROUP_SIZE]),
                        sv,
                        op=Alu.add,
                    )
                    # s2 carries bias in the low bits: s2 = sshift + EPS*bias
                    s2 = rpool.tile([128, E], f32, tag="s2")
                    nc.gpsimd.tensor_tensor(s2[:], bes[:], sshift[:], op=Alu.add)

                    v8 = spool.tile([128, TOPK], f32, tag="v8")
                    nc.vector.max(v8[:], sshift[:])
                    nc.vector.max_index(iraw[:, t, :], v8[:], sshift[:])
                    v8b = spool.tile([128, TOPK], f32, tag="v8b")
                    nc.vector.max(v8b[:], s2[:])
                    # sig[idx_j] = (v8_j - SHIFT) - (v8b_j - v8_j)/EPS
                    dd = spool.tile([128, TOPK], f32, tag="dd")
                    nc.vector.tensor_tensor(dd[:], v8b[:], v8[:], op=Alu.subtract)
                    v8m = spool.tile([128, TOPK], f32, tag="v8m")
                    nc.vector.tensor_scalar(
                        v8m[:], v8[:], -SHIFT, None, op0=Alu.add
                    )
                    nc.vector.scalar_tensor_tensor(
                        sig_top[:, t, :], dd[:], -1.0 / EPS, v8m[:],
                        op0=Alu.mult, op1=Alu.add,
                    )

                # ---- per-super-tile finalization (2 token tiles) ----
                stv = sig_top[:, st * 2:st * 2 + 2, :]
                ssum = spool.tile([128, 2], f32, tag="ssum")
                nc.vector.reduce_sum(ssum[:], stv, axis=AX.X)
                rec = spool.tile([128, 2], f32, tag="rec")
                nc.vector.reciprocal(rec[:], ssum[:])
                wres = spool.tile([128, 2, TOPK], f32, tag="wres")
                nc.vector.scalar_tensor_tensor(
                    wres[:],
                    stv,
                    ROUTE_SCALE,
                    rec[:].broadcast_to([128, 2, TOPK]),
                    op0=Alu.mult,
                    op1=Alu.mult,
                )
                icast = spool.tile([128, 2, TOPK], u32, tag="icast")
                nc.vector.tensor_copy(icast[:], iraw[:, st * 2:st * 2 + 2, :])
                nc.sync.dma_start(
                    wout[:, st * 2 * TOPK:(st + 1) * 2 * TOPK],
                    wres[:].rearrange("p a b -> p (a b)"),
                )
                nc.sync.dma_start(
                    iout[:, st * 2 * TOPK:(st + 1) * 2 * TOPK],
                    icast[:].rearrange("p a b -> p (a b)"),
                )

    nc.compile()
    return nc


def _get_nc():
    global _NC
    if _NC is None:
        _NC = _build_nc()
    return _NC


def kernel(x: np.ndarray, weight: np.ndarray, bias: np.ndarray):
    from concourse import bass_utils

    nc = _get_nc()

    wt = np.ascontiguousarray(weight.astype(np.float32).T).astype(np.float16)
    wp = np.ascontiguousarray(
        wt.reshape(KT, 128, E).transpose(1, 0, 2).reshape(128, KT * E)
    )
    bb = np.ascontiguousarray(np.tile(bias.astype(np.float32), (128, 1)))
    be = np.ascontiguousarray(bb * np.float32(EPS))

    in_maps = []
    for i in range(N_CORES):
        xs = x[i * T:(i + 1) * T]
        xt = np.ascontiguousarray(xs.T).astype(np.float16)      # [D, T]
        xpk = np.ascontiguousarray(
            xt.reshape(KT, 128, NST, 256).transpose(2, 1, 0, 3)
            .reshape(NST * 128, KT * 256)
        )
        in_maps.append({"xp": xpk, "wp": wp, "bb": bb, "be": be})

    res = bass_utils.run_bass_kernel_spmd(nc, in_maps, core_ids=list(range(N_CORES)))

    w_parts, i_parts = [], []
    for r in res.results:
        wv = r["wout"].reshape(128, NT, TOPK).transpose(1, 0, 2).reshape(T, TOPK)
        iv = r["iout"].reshape(128, NT, TOPK).transpose(1, 0, 2).reshape(T, TOPK)
        w_parts.append(wv)
        i_parts.append(iv.astype(np.int32))
    return np.concatenate(w_parts, 0), np.concatenate(i_parts, 0)


# revision 14
# speedup vs baseline: 1.0096x; 1.0096x over previous
import os
import sys

for _p in ("/opt/trn_rl_repo", "/root/.axon_site/_ro/trn_rl_repo"):
    if os.path.isdir(_p) and _p not in sys.path:
        sys.path.insert(0, _p)

import numpy as np


def _ensure_ntff_hook():
    """bass_utils hard-imports antenv.axon_hooks when BASS_TRACE is set.
    The agent image ships an antenv stub without it; register the same
    ctypes-based hook trn_boot would have installed."""
    try:
        import antenv.axon_hooks  # noqa: F401
        return
    except ImportError:
        pass
    import types
    import ctypes
    import contextlib

    mod = types.ModuleType("antenv.axon_hooks")
    state = {"hook": None}

    def set_axon_ntff_profile_hook(h):
        state["hook"] = h

    def get_axon_ntff_profile_hook():
        return state["hook"]

    mod.set_axon_ntff_profile_hook = set_axon_ntff_profile_hook
    mod.get_axon_ntff_profile_hook = get_axon_ntff_profile_hook

    so_path = "/opt/axon/libaxon_pjrt.so"
    try:
        lib = ctypes.CDLL(so_path)
        if hasattr(lib, "axon_start_nrt_profile"):
            lib.axon_start_nrt_profile.argtypes = [
                ctypes.POINTER(ctypes.c_int64),
                ctypes.c_size_t,
            ]
            lib.axon_start_nrt_profile.restype = ctypes.c_int64
            lib.axon_stop_nrt_profile.argtypes = [ctypes.c_char_p]
            lib.axon_stop_nrt_profile.restype = ctypes.c_int64

            @contextlib.contextmanager
            def _hook(output_dir, device_ids):
                import jax

                jax.devices()
                if device_ids:
                    ids = (ctypes.c_int64 * len(device_ids))(*device_ids)
                    rc = lib.axon_start_nrt_profile(ids, len(device_ids))
                else:
                    rc = lib.axon_start_nrt_profile(None, 0)
                if rc != 0:
                    raise RuntimeError(f"axon_start_nrt_profile rc={rc}")
                try:
                    yield
                finally:
                    n = lib.axon_stop_nrt_profile(str(output_dir).encode())
                    if n < 0:
                        raise RuntimeError(f"axon_stop_nrt_profile rc={n}")

            state["hook"] = _hook
    except OSError:
        pass

    import antenv

    antenv.axon_hooks = mod
    sys.modules["antenv.axon_hooks"] = mod


_ensure_ntff_hook()

N_CORES = 8
T_FULL = 16384
T = T_FULL // N_CORES      # 2048 tokens per core
D = 7168
E = 256
KT = D // 128              # 56 contraction tiles
NT = T // 128              # 16 token tiles per core
NST = 8                    # super-tiles of 256 tokens
CS = [4, 12, 20, 20]       # k-tiles per DMA chunk (small first => early PE start)
CO = [0, 4, 16, 36]        # chunk k-tile offsets
NCHUNK = len(CS)

N_GROUPS = 8
GROUP_SIZE = E // N_GROUPS  # 32
TOPK = 8
ROUTE_SCALE = 2.5
SHIFT = 4.0                # group-select shift: separates selected groups
EPS = 2.0 ** -11           # bias-encoding scale for the second top-8 pass

_NC = None


def _build_nc():
    import concourse.bass as bass
    import concourse.tile as tile
    from concourse import bacc, mybir

    nc = bacc.Bacc(None, target_bir_lowering=False)
    f32 = mybir.dt.float32
    f16 = mybir.dt.float16
    u16 = mybir.dt.uint16
    u32 = mybir.dt.uint32
    Alu = mybir.AluOpType
    AX = mybir.AxisListType
    Act = mybir.ActivationFunctionType

    # host-packed layouts (see kernel()):
    #   xp[st*128+p, k*256+c] = x[token st*256+c, dim k*128+p]  (fp16)
    #   wp[p, k*256+e]        = weight[e, k*128+p]              (fp16)
    xp = nc.dram_tensor("xp", [NST * 128, KT * 256], f16, kind="ExternalInput")
    wp = nc.dram_tensor("wp", [128, KT * 256], f16, kind="ExternalInput")
    bb = nc.dram_tensor("bb", [128, E], f32, kind="ExternalInput")
    be = nc.dram_tensor("be", [128, E], f32, kind="ExternalInput")  # bias * EPS
    wout = nc.dram_tensor("wout", [128, NT * TOPK], f32, kind="ExternalOutput")
    iout = nc.dram_tensor("iout", [128, NT * TOPK], u32, kind="ExternalOutput")

    with tile.TileContext(nc) as tc:
        with (
            tc.tile_pool(name="const", bufs=1) as cpool,
            tc.tile_pool(name="xc", bufs=3) as xpool,
            tc.tile_pool(name="rt", bufs=3) as rpool,
            tc.tile_pool(name="sm", bufs=3) as spool,
            tc.tile_pool(name="ps", bufs=4, space=bass.MemorySpace.PSUM) as pspool,
        ):
            # resident gate weight [128, KT*256] fp16, loaded in 4 chunks
            wsb = cpool.tile([128, KT * 256], f16)
            bbs = cpool.tile([128, E], f32)
            bes = cpool.tile([128, E], f32)

            # persistent accumulators
            sig_top = cpool.tile([128, NT, TOPK], f32)
            iraw = cpool.tile([128, NT, TOPK], u16)

            for st in range(NST):
                chunks = []
                for c in range(NCHUNK):
                    lo, hi = CO[c] * 256, (CO[c] + CS[c]) * 256
                    if st == 0:
                        # w chunks on the Sync HWDGE queue, x chunks on the
                        # Scalar queue: descriptor generation (~0.7us each)
                        # runs in parallel and the first w/x pair lands first
                        nc.sync.dma_start(wsb[:, lo:hi], wp[:, lo:hi])
                    xc = xpool.tile([128, CS[c] * 256], f16, tag=f"xc{c}")
                    nc.scalar.dma_start(
                        xc[:], xp[st * 128:(st + 1) * 128, lo:hi]
                    )
                    chunks.append(xc)
                if st == 0:
                    nc.sync.dma_start(bbs[:], bb[:, :])
                    nc.sync.dma_start(bes[:], be[:, :])

                for tt in range(2):
                    t = st * 2 + tt
                    ps = pspool.tile([128, E], f32)
                    for k in range(KT):
                        c = max(i for i in range(NCHUNK) if CO[i] <= k)
                        off = (k - CO[c]) * 256 + tt * 128
                        nc.tensor.matmul(
                            ps[:],
                            chunks[c][:, off:off + 128],
                            wsb[:, k * 256:(k + 1) * 256],
                            start=(k == 0),
                            stop=(k == KT - 1),
                        )

                    # ---- routing for this 128-token tile ----
                    # adds run on the idle GpSimd engine, except for the last
                    # super-tile where the shorter all-DVE chain cuts the tail
                    eng = nc.vector if st == NST - 1 else nc.gpsimd
                    sig = rpool.tile([128, E], f32, tag="sig")
                    nc.scalar.activation(sig[:], ps[:], Act.Sigmoid)
                    s = rpool.tile([128, E], f32, tag="s")
                    eng.tensor_tensor(s[:], sig[:], bbs[:], op=Alu.add)
                    sv = s[:].rearrange("p (g e) -> p g e", g=N_GROUPS)

                    m1 = spool.tile([128, N_GROUPS], f32, tag="m1")
                    nc.vector.reduce_max(m1[:], sv, axis=AX.X)
                    mr = rpool.tile([128, E], f32, tag="mr")
                    nc.vector.match_replace(mr[:], m1[:], s[:], -1e30)
                    m2 = spool.tile([128, N_GROUPS], f32, tag="m2")
                    nc.vector.reduce_max(
                        m2[:], mr[:].rearrange("p (g e) -> p g e", g=N_GROUPS),
                        axis=AX.X,
                    )
                    gs = spool.tile([128, N_GROUPS], f32, tag="gs")
                    nc.vector.tensor_tensor(gs[:], m1[:], m2[:], op=Alu.add)
                    srt = spool.tile([128, N_GROUPS], f32, tag="srt")
                    nc.vector.max(srt[:], gs[:])
                    # keep4 = (gs >= 4th-largest) * SHIFT
                    keep4 = spool.tile([128, N_GROUPS], f32, tag="keep4")
                    nc.vector.tensor_scalar(
                        keep4[:], gs[:], srt[:, 3:4], SHIFT,
                        op0=Alu.is_ge, op1=Alu.mult,
                    )
                    sshift = rpool.tile([128, E], f32, tag="sshift")
                    eng.tensor_tensor(
                        sshift[:].rearrange("p (g e) -> p g e", g=N_GROUPS),
                        keep4[:].broadcast_to([128, N_GROUPS, GROUP_SIZE]),
                        sv,
                        op=Alu.add,
                    )
                    # s2 carries bias in the low bits: s2 = sshift + EPS*bias
                    s2 = rpool.tile([128, E], f32, tag="s2")
                    eng.tensor_tensor(s2[:], bes[:], sshift[:], op=Alu.add)

                    v8 = spool.tile([128, TOPK], f32, tag="v8")
                    nc.vector.max(v8[:], sshift[:])
                    nc.vector.max_index(iraw[:, t, :], v8[:], sshift[:])
                    v8b = spool.tile([128, TOPK], f32, tag="v8b")
                    nc.vector.max(v8b[:], s2[:])
                    # sig[idx_j] = (v8_j - SHIFT) - (v8b_j - v8_j)/EPS
                    dd = spool.tile([128, TOPK], f32, tag="dd")
                    nc.vector.tensor_tensor(dd[:], v8b[:], v8[:], op=Alu.subtract)
                    v8m = spool.tile([128, TOPK], f32, tag="v8m")
                    nc.vector.tensor_scalar(
                        v8m[:], v8[:], -SHIFT, None, op0=Alu.add
                    )
                    nc.vector.scalar_tensor_tensor(
                        sig_top[:, t, :], dd[:], -1.0 / EPS, v8m[:],
                        op0=Alu.mult, op1=Alu.add,
                    )

                # ---- per-super-tile finalization (2 token tiles) ----
                stv = sig_top[:, st * 2:st * 2 + 2, :]
                ssum = spool.tile([128, 2], f32, tag="ssum")
                nc.vector.reduce_sum(ssum[:], stv, axis=AX.X)
                rec = spool.tile([128, 2], f32, tag="rec")
                nc.vector.reciprocal(rec[:], ssum[:])
                wres = spool.tile([128, 2, TOPK], f32, tag="wres")
                nc.vector.scalar_tensor_tensor(
                    wres[:],
                    stv,
                    ROUTE_SCALE,
                    rec[:].broadcast_to([128, 2, TOPK]),
                    op0=Alu.mult,
                    op1=Alu.mult,
                )
                icast = spool.tile([128, 2, TOPK], u32, tag="icast")
                nc.vector.tensor_copy(icast[:], iraw[:, st * 2:st * 2 + 2, :])
                nc.sync.dma_start(
                    wout[:, st * 2 * TOPK:(st + 1) * 2 * TOPK],
                    wres[:].rearrange("p a b -> p (a b)"),
                )
                nc.sync.dma_start(
                    iout[:, st * 2 * TOPK:(st + 1) * 2 * TOPK],
                    icast[:].rearrange("p a b -> p (a b)"),
                )

    nc.compile()
    return nc


def _get_nc():
    global _NC
    if _NC is None:
        _NC = _build_nc()
    return _NC


def kernel(x: np.ndarray, weight: np.ndarray, bias: np.ndarray):
    from concourse import bass_utils

    nc = _get_nc()

    wt = np.ascontiguousarray(weight.astype(np.float32).T).astype(np.float16)
    wp = np.ascontiguousarray(
        wt.reshape(KT, 128, E).transpose(1, 0, 2).reshape(128, KT * E)
    )
    bb = np.ascontiguousarray(np.tile(bias.astype(np.float32), (128, 1)))
    be = np.ascontiguousarray(bb * np.float32(EPS))

    in_maps = []
    for i in range(N_CORES):
        xs = x[i * T:(i + 1) * T]
        xt = np.ascontiguousarray(xs.T).astype(np.float16)      # [D, T]
        xpk = np.ascontiguousarray(
            xt.reshape(KT, 128, NST, 256).transpose(2, 1, 0, 3)
            .reshape(NST * 128, KT * 256)
        )
        in_maps.append({"xp": xpk, "wp": wp, "bb": bb, "be": be})

    res = bass_utils.run_bass_kernel_spmd(nc, in_maps, core_ids=list(range(N_CORES)))

    w_parts, i_parts = [], []
    for r in res.results:
        wv = r["wout"].reshape(128, NT, TOPK).transpose(1, 0, 2).reshape(T, TOPK)
        iv = r["iout"].reshape(128, NT, TOPK).transpose(1, 0, 2).reshape(T, TOPK)
        w_parts.append(wv)
        i_parts.append(iv.astype(np.int32))
    return np.concatenate(w_parts, 0), np.concatenate(i_parts, 0)
